# revision 1
# baseline (speedup 1.0000x reference)
# Trainium2 Bass kernel for nn_AutoRegressive (LSTM warmup + autoregressive decode).
#
# Problem: B=512, T=128, F=64, UNITS=1024, OUT_STEPS=32.
#   warmup: 128 sequential LSTM steps over inputs, keep final (h, c)
#   decode: pred = h @ Wd + bd, feed pred back as x for 31 more steps
#   output: [B, 32, F]
#
# Strategy: pure 8-way data parallelism on the batch axis (64 rows/core),
# weights replicated, zero cross-core communication. Per step the dominant
# matmul z = x @ Wk + h @ Wr is computed with h^T-stationary matmuls
# (lhsT = h^T[k-chunk] [128, 64]) streaming Wr columns. Because the local
# batch is 64 (< 128 array columns), two matmuls are column-tiled at
# tile_position (0,0)/(0,64) to process the lo/hi unit-halves of each gate
# concurrently (emitted adjacently so the PE overlaps them), keeping the
# 128x128 PE array fully utilized.
# All matmul operands are bf16 (PSUM accumulates f32); gates/state are f32.
# h -> h^T via 4 PE transposes per step; each transpose PAIR has its own
# PSUM bank and its own hT SBUF tile (hTa/hTb), and the next step's k-loop
# is ordered by chunk readiness so the pair-0 copy unblocks half of it
# while pair 1 is in flight. The g and o gates are split into 2x256-col
# PSUM tiles so the c/h gate chain pipelines with the matmul tail; zero-
# accumulate filler matmuls absorb the remaining PE wait windows (keeps
# the HAM clock gate at 8/8). Decode runs h@Wr first and x@Wk last so the
# pred -> x_dec chain hides under the matmuls; pred copies run on ScalarE
# with bd folded in as a per-partition Identity bias. Bias b is folded
# into an augmented ones-row of x / extra row of Wk on the host.
# Measured: 1.811 ms exec on hardware, rel err 3.4e-3 vs the reference.
import os
import sys

sys.path.insert(0, "/opt/trn_rl_repo")

import numpy as np
import ml_dtypes

import concourse.bass as bass
import concourse.mybir as mybir
import concourse.tile as tile
from concourse import bacc
from concourse.bass_utils import run_bass_kernel_spmd
from concourse.masks import make_identity
from contextlib import ExitStack

F32, BF16 = mybir.dt.float32, mybir.dt.bfloat16
AF = mybir.ActivationFunctionType
Alu = mybir.AluOpType

B_FULL, T_FULL, F_DIM, UNITS = 512, 128, 64, 1024
N_CORES = 8
B = B_FULL // N_CORES          # 64 local batch rows
NK = UNITS // 128              # 8 k-chunks of the recurrent contraction
GATES = [1, 0, 2, 3]           # processing order f,i,g,o (orig packing i,f,g,o)

_NC_CACHE = {}


def _build(n_warm: int, n_out: int):
    """Build the per-core Bass program. n_out = number of predictions (32)."""
    key = (n_warm, n_out)
    if key in _NC_CACHE:
        return _NC_CACHE[key]

    n_dec = n_out - 1  # LSTM steps in decode phase

    nc = bacc.Bacc("TRN2", target_bir_lowering=False, debug=False,
                   num_devices=N_CORES)
    xt_ext = nc.dram_tensor("xt", [n_warm, F_DIM + 1, B], BF16,
                            kind="ExternalInput")
    wr_ext = nc.dram_tensor("wr", [128, NK * 8 * 512], BF16,
                            kind="ExternalInput")
    wk_ext = nc.dram_tensor("wk", [F_DIM + 1, 8 * 512], BF16,
                            kind="ExternalInput")
    wd_ext = nc.dram_tensor("wd", [128, NK * F_DIM], BF16,
                            kind="ExternalInput")
    bd_ext = nc.dram_tensor("bd", [F_DIM, 1], F32, kind="ExternalInput")
    out_ext = nc.dram_tensor("out", [F_DIM, n_out * B], F32,
                             kind="ExternalOutput")

    with ExitStack() as ctx:
        tc = ctx.enter_context(tile.TileContext(nc))
        wpool = ctx.enter_context(tc.tile_pool(name="w", bufs=1))
        xpool = ctx.enter_context(tc.tile_pool(name="x", bufs=3))
        hTpool = ctx.enter_context(tc.tile_pool(name="hT", bufs=2))
        hpool = ctx.enter_context(tc.tile_pool(name="h", bufs=2))
        cpool = ctx.enter_context(tc.tile_pool(name="c", bufs=2))
        gpool = ctx.enter_context(tc.tile_pool(name="g", bufs=2))
        xdpool = ctx.enter_context(tc.tile_pool(name="xd", bufs=2))
        zpool = ctx.enter_context(tc.tile_pool(name="z", bufs=1, space="PSUM"))
        zspool = ctx.enter_context(tc.tile_pool(name="zs", bufs=2, space="PSUM"))
        tpool = ctx.enter_context(tc.tile_pool(name="tp", bufs=1, space="PSUM"))

        wr_sb = wpool.tile([128, NK * 8 * 512], BF16)
        nc.sync.dma_start(wr_sb[:], wr_ext[:])
        wk_sb = wpool.tile([F_DIM + 1, 8 * 512], BF16)
        nc.sync.dma_start(wk_sb[:], wk_ext[:])
        wd_sb = wpool.tile([128, NK * F_DIM], BF16)
        nc.sync.dma_start(wd_sb[:], wd_ext[:])
        bd_sb = wpool.tile([F_DIM, 1], F32)
        nc.sync.dma_start(bd_sb[:], bd_ext[:])
        identb = wpool.tile([128, 128], BF16)
        make_identity(nc, identb[:])
        ones_sb = wpool.tile([1, B], BF16)
        nc.vector.memset(ones_sb[:], 1.0)
        zeros_sb = wpool.tile([128, 512], BF16)
        nc.vector.memset(zeros_sb[:], 0.0)
        preds_sb = wpool.tile([F_DIM, n_out * B], F32)
        xd_sb = wpool.tile([F_DIM + 1, B], BF16)
        nc.vector.memset(xd_sb[F_DIM:F_DIM + 1, :], 1.0)

        state = {"h0": None, "h1": None, "c": None, "hT": None}
        # hT column layout: transpose of h[:, j*128:(j+1)*128] yields unit
        # chunks j (cols 0:64) and j+4 (cols 64:128); store them adjacently
        # so each transpose pair needs ONE contiguous DVE copy.
        HT_POS = {}
        for j in range(4):
            HT_POS[j] = 2 * j
            HT_POS[j + 4] = 2 * j + 1

        def hT_sl(k):
            p = HT_POS[k]
            t = state["hTa"] if p < 4 else state["hTb"]
            return t[:, (p % 4) * B:(p % 4 + 1) * B]

        def transposes(js):
            """h halves (bf16, batch-major split layout) -> hT chunks (bf16).

            transpose of h[:, j*128:(j+1)*128] yields unit chunks j and j+4
            side by side. Each pair gets its own PSUM bank and its own hT
            SBUF tile so the copy of pair 0 (chunks 0,4,1,5) unblocks the
            first half of the next step's k-loop while pair 1 is still in
            flight."""
            # Each transpose pair gets its OWN PSUM bank and its own hT
            # SBUF tile: the pair-0 copy (chunks 0,4,1,5) unblocks the first
            # half of the next step's k-loop while pair 1 and its copy are
            # still in flight — real work instead of filler in the
            # HAM-activity window.
            half = js[0] // 2
            tag = "tp0" if half == 0 else "tps"
            tps = tpool.tile([128, 1024], BF16, name=tag, tag=tag)[:, 0:256]
            hT = state["hTa"] if half == 0 else state["hTb"]
            for jj, j in enumerate(js):
                h_half = state["h0"] if j < 2 else state["h1"]
                nc.tensor.transpose(tps[:, jj * 128:(jj + 1) * 128],
                                    h_half[:, (j % 2) * 128:(j % 2 + 1) * 128],
                                    identb[:])
            nc.vector.tensor_copy(hT[:], tps[:])

        def keep_warm(zs, n, start=False):
            """Filler matmuls accumulating an all-zeros K=1 product into the
            live zf tile: numerically a no-op, but real PE activity (keeps
            the HAM clock gate at 8/8 across the per-step gate-chain tail)
            that writes a consumed tile (so DCE keeps it). With start=True
            the first one opens zf's group (decode, where x@Wk comes last)."""
            zf = zs[0]
            for i in range(n):
                nc.tensor.matmul(zf[0:64, :], wr_sb[:, 0:64], zeros_sb[:],
                                 start=(start and i == 0), stop=False)

        def pred_block(d):
            """pred_d^T = Wd^T @ h + bd from current hT; returns x_dec tile."""
            # share the pair-0 transpose bank: it is released right after
            # copy-a (early), so the pred matmuls start ~0.7us sooner than
            # waiting for copy-b's slot. Safe now that the pred copies run
            # on ScalarE (the old DVE-queue release stall is gone).
            pp = tpool.tile([F_DIM, 512], F32, name="pp", tag="tp0")[:, 0:B]
            for ki, k in enumerate((0, 4, 1, 5, 2, 6, 3, 7)):
                nc.tensor.matmul(pp[:], wd_sb[:, k * F_DIM:(k + 1) * F_DIM],
                                 hT_sl(k), start=(k == 0), stop=(ki == 7))
            # Copies on ScalarE (idle here, and off the DVE queue which is
            # busy with the hT copies); bd is per-partition on pred^T so it
            # folds into the copy as an Identity bias — this also kills the
            # pathologically slow K=1 bd matmul (~630ns) from the chain.
            nc.scalar.activation(preds_sb[:, d * B:(d + 1) * B], pp[:],
                                 AF.Identity, bias=bd_sb[:])
            if d < n_out - 1:
                nc.scalar.activation(xd_sb[0:F_DIM, :], pp[:],
                                     AF.Identity, bias=bd_sb[:])
                return xd_sb
            return None

        def alloc_z():
            """Gate order f, i, g, o; g and o split into two 256-col chunks
            in separate PSUM tiles (full [128,512] banks, first 256 cols
            used — half-bank tiles would share banks and the bank-overlap
            tracker serializes reads against the bank-mate's writes)."""
            zf = zpool.tile([128, 512], F32, name="zf", tag="zf")
            zi = zpool.tile([128, 512], F32, name="zi", tag="zi")
            zg = [zspool.tile([128, 512], F32, name="zg%d" % s, tag="zg")[:, 0:256]
                  for s in (0, 1)]
            zo = [zspool.tile([128, 512], F32, name="zo%d" % s, tag="zo")[:, 0:256]
                  for s in (0, 1)]
            return (zf, zi, zg, zo)

        def z_layout(zs):
            zf, zi, zg, zo = zs
            return ((0, zf, 0, 512), (1, zi, 0, 512),
                    (2, zg[0], 0, 256), (2, zg[1], 256, 256),
                    (3, zo[0], 0, 256), (3, zo[1], 256, 256))

        def emit_xwk(zs, x_sb, start, stop, blocks=None):
            """x @ Wk (+b); opens the PSUM groups when start=True (warmup)
            or closes them when stop=True (decode, where it comes last).
            `blocks` selects a subset of z_layout entries (warmup emits the
            o blocks separately: they wait on the previous step's sig_o
            PSUM-slot release, and fillers must cover that window)."""
            lay = z_layout(zs)
            if blocks is not None:
                lay = [lay[b] for b in blocks]
            for blk, z, lo, n in lay:
                for half in (0, 1):
                    o = (half * 4 + blk) * 512 + lo
                    nc.tensor.matmul(z[half * 64:(half + 1) * 64, :],
                                     x_sb[:], wk_sb[:, o:o + n],
                                     start=start, stop=stop)

        def emit_hwr(zs, xwk_first):
            """h @ Wr — lo/hi column-tile pairs emitted adjacently; k-chunks
            ordered by hT readiness. When xwk_first is False (decode), this
            opens the groups (except zf, opened by keep_warm) and leaves
            them open for the trailing x@Wk."""
            zf = zs[0]
            for blk, z, lo, n in z_layout(zs):
                for ki, k in enumerate((0, 4, 1, 5, 2, 6, 3, 7)):
                    stop = xwk_first and k == NK - 1
                    for half in (0, 1):
                        # keep_warm only opened zf's lo half (partitions 0:64)
                        start = ((not xwk_first) and ki == 0
                                 and not (z is zf and half == 0))
                        o = (k * 8 + half * 4 + blk) * 512 + lo
                        nc.tensor.matmul(
                            z[half * 64:(half + 1) * 64, :],
                            hT_sl(k), wr_sb[:, o:o + n],
                            start=start, stop=stop)

        def lstm_rest(zs, first):
            zf, zi, zg, zo = zs
            # gates: per-half SBUF tiles so halves pipeline independently
            c_prev = state["c"]
            sig_f = None
            if not first:
                sig_f = gpool.tile([128, 512], F32, tag="sig_f", name="sig_f")
                nc.scalar.activation(sig_f[:], zf[:], AF.Sigmoid)
            sig_i = gpool.tile([128, 512], F32, tag="sig_i", name="sig_i")
            nc.scalar.activation(sig_i[:], zi[:], AF.Sigmoid)
            c_new, h_new = [], []
            tanh_gs = []
            for s in (0, 1):
                tg = gpool.tile([128, 256], F32, tag="tanh_g%d" % s,
                                name="tanh_g")
                nc.scalar.activation(tg[:], zg[s][:], AF.Tanh)
                tanh_gs.append(tg)
            for s in (0, 1):
                sl = slice(s * 256, (s + 1) * 256)
                cs = cpool.tile([128, 256], F32, tag="c%d" % s, name="c")
                if first:
                    nc.vector.tensor_tensor(cs[:], sig_i[:, sl], tanh_gs[s][:],
                                            Alu.mult)
                else:
                    t1 = gpool.tile([128, 256], F32, tag="t1_%d" % s, name="t1")
                    nc.vector.tensor_tensor(t1[:], sig_i[:, sl], tanh_gs[s][:],
                                            Alu.mult)
                    nc.vector.tensor_tensor(cs[:], sig_f[:, sl], c_prev[s][:],
                                            Alu.mult)
                    nc.vector.tensor_tensor(cs[:], cs[:], t1[:], Alu.add)
                # sig_o before tanh_c: frees the zo PSUM slot as soon as
                # the o-half matmuls finish (the next step's x@Wk-o waits on
                # it), and h can fire right after tanh_c.
                so = gpool.tile([128, 256], F32, tag="sig_o%d" % s,
                                name="sig_o")
                nc.scalar.activation(so[:], zo[s][:], AF.Sigmoid)
                tc_s = gpool.tile([128, 256], F32, tag="tanh_c%d" % s,
                                  name="tanh_c")
                nc.scalar.activation(tc_s[:], cs[:], AF.Tanh)
                hs = hpool.tile([128, 256], BF16, tag="h%d" % s, name="h")
                nc.vector.tensor_tensor(hs[:], so[:], tc_s[:], Alu.mult)
                c_new.append(cs)
                h_new.append(hs)
            state["h0"], state["h1"], state["c"] = h_new[0], h_new[1], c_new

        # ---- warmup ----
        for t in range(n_warm):
            x_sb = xpool.tile([F_DIM + 1, B], BF16)
            nc.sync.dma_start(x_sb[:], xt_ext[t])
            zs = alloc_z()
            if t == 0:
                emit_xwk(zs, x_sb, start=True, stop=True)
            else:
                emit_xwk(zs, x_sb, start=True, stop=False, blocks=(0, 1, 2, 3))
                keep_warm(zs, 2)
                emit_xwk(zs, x_sb, start=True, stop=False, blocks=(4, 5))
                keep_warm(zs, 2)
                state["hTa"] = hTpool.tile([128, 4 * B], BF16, name="hTa", tag="hTa")
                state["hTb"] = hTpool.tile([128, 4 * B], BF16, name="hTb", tag="hTb")
                transposes([0, 1])
                transposes([2, 3])
                emit_hwr(zs, xwk_first=True)
            lstm_rest(zs, first=(t == 0))

        # ---- decode: h@Wr first, x@Wk last, so the pred -> x_dec chain
        # hides under the recurrent matmuls ----
        for d in range(n_dec):
            zs = alloc_z()
            keep_warm(zs, 3, start=True)
            state["hTa"] = hTpool.tile([128, 4 * B], BF16, name="hTa", tag="hTa")
            state["hTb"] = hTpool.tile([128, 4 * B], BF16, name="hTb", tag="hTb")
            transposes([0, 1])
            transposes([2, 3])
            keep_warm(zs, 2)
            xd = pred_block(d)
            emit_hwr(zs, xwk_first=False)
            emit_xwk(zs, xd, start=False, stop=True)
            lstm_rest(zs, False)
        state["hTa"] = hTpool.tile([128, 4 * B], BF16, name="hTa", tag="hTa")
        state["hTb"] = hTpool.tile([128, 4 * B], BF16, name="hTb", tag="hTb")
        transposes([0, 1])
        transposes([2, 3])
        pred_block(n_out - 1)

        nc.sync.dma_start(out_ext[:], preds_sb[:])

    nc.finalize()
    _NC_CACHE[key] = nc
    return nc


def _prep_core_inputs(inputs, Wk, Wr, b, Wd, bd, n_warm, n_out):
    """Host-side reshaping/sharding. Returns list of 8 input dicts."""
    bf = lambda a: np.ascontiguousarray(a).astype(ml_dtypes.bfloat16)
    perm = np.array([g * UNITS + hh * 512 + k
                     for hh in (0, 1) for g in GATES for k in range(512)])
    Wk_aug = np.concatenate([Wk, b[None, :]], 0)[:, perm]        # [65, 4096]
    Wr_p = Wr[:, perm]                                           # [1024, 4096]
    wr_dev = bf(np.stack([Wr_p[k * 128:(k + 1) * 128] for k in range(NK)],
                         1).reshape(128, -1))
    wk_dev = bf(Wk_aug)
    wd_dev = bf(np.stack([Wd[k * 128:(k + 1) * 128] for k in range(NK)],
                         1).reshape(128, -1))
    bd_dev = np.ascontiguousarray(bd[:, None]).astype(np.float32)

    in_maps = []
    for c in range(N_CORES):
        xs = inputs[c * B:(c + 1) * B, :n_warm]                  # [64, T, F]
        xt = xs.transpose(1, 2, 0)                               # [T, F, 64]
        xt_aug = np.concatenate(
            [xt, np.ones((n_warm, 1, B), np.float32)], 1)        # [T, 65, 64]
        in_maps.append({
            "xt": bf(xt_aug), "wr": wr_dev, "wk": wk_dev,
            "wd": wd_dev, "bd": bd_dev,
        })
    return in_maps


def kernel(inputs, Wk, Wr, b, Wd, bd, out_steps):
    inputs = np.asarray(inputs, np.float32)
    Wk = np.asarray(Wk, np.float32)
    Wr = np.asarray(Wr, np.float32)
    b = np.asarray(b, np.float32)
    Wd = np.asarray(Wd, np.float32)
    bd = np.asarray(bd, np.float32)
    n_out = int(out_steps)
    n_warm = inputs.shape[1]

    nc = _build(n_warm, n_out)
    in_maps = _prep_core_inputs(inputs, Wk, Wr, b, Wd, bd, n_warm, n_out)
    res = run_bass_kernel_spmd(nc, in_maps, core_ids=list(range(N_CORES)))

    out = np.empty((B_FULL, n_out, F_DIM), np.float32)
    for c in range(N_CORES):
        o = res.results[c]["out"].reshape(F_DIM, n_out, B)       # [F, t, b]
        out[c * B:(c + 1) * B] = o.transpose(2, 1, 0)
    return out



# revision 16
# speedup vs baseline: 1.3074x; 1.3074x over previous
# Trainium2 Bass kernel for nn_AutoRegressive (LSTM warmup + autoregressive decode).
#
# Problem: B=512, T=128, F=64, UNITS=1024, OUT_STEPS=32.
#   warmup: 128 sequential LSTM steps over inputs, keep final (h, c)
#   decode: pred = h @ Wd + bd, feed pred back as x for 31 more steps
#   output: [B, 32, F]
#
# Strategy: pure 8-way data parallelism on the batch axis (64 rows/core),
# weights replicated, zero cross-core communication. Per step the dominant
# matmul z = x @ Wk + h @ Wr is computed with h^T-stationary matmuls
# (lhsT = h^T[k-chunk] [128, 64]) streaming Wr columns. Because the local
# batch is 64 (< 128 array columns), each matmul pair is column-tiled at
# (0,0)/(0,64) covering the lo/hi unit-halves of a gate concurrently.
#
# v2 restructure vs the 1.85ms baseline:
#  - The 4 gates are emitted as EIGHT 256-column runs (f0,f1,i0,i1,g0,g1,
#    o0,o1), each run = its x@Wk pair followed by its full 8-chunk h@Wr
#    k-loop. Gate slices therefore complete progressively through the PE
#    stream and the sigmoid/tanh chain runs concurrently with the matmuls
#    instead of serially at the step tail (was ~3.4us of tail).
#  - Gate/cell elementwise chain in fp16 (c state, sigmoids, tanh) and
#    bf16 (h): 2-byte dtypes + all-SBUF operands put the DVE in its 4x
#    mode (~510ns -> ~130ns per [128,256] op).
#  - Engines are FIFO: ACT/DVE ops are emitted in exact readiness order
#    (sig_f0..sig_o1 / c-chain) so nothing blocks the h tail.
#  - Transposes of h are split around the first run: pair-0 (+copy) right
#    after the previous step's o1 run, then the first 4 k-chunk pairs of
#    run f0 execute while pair-1 transposes+copy complete.
#  - Filler matmuls dropped: PE gaps are now well under the ~3.4us HAM
#    re-throttle window.
# Bias b is folded into an augmented ones-row of x / extra row of Wk.
# pred copies run on ScalarE with bd folded in as an Identity bias.
import os
import sys

sys.path.insert(0, "/opt/trn_rl_repo")

import numpy as np
import ml_dtypes

import concourse.bass as bass
import concourse.mybir as mybir
import concourse.tile as tile
from concourse import bacc
from concourse.bass_utils import run_bass_kernel_spmd
from concourse.masks import make_identity
from contextlib import ExitStack

F32, F16, BF16 = mybir.dt.float32, mybir.dt.float16, mybir.dt.bfloat16
AF = mybir.ActivationFunctionType
Alu = mybir.AluOpType

B_FULL, T_FULL, F_DIM, UNITS = 512, 128, 64, 1024
N_CORES = 8
B = B_FULL // N_CORES          # 64 local batch rows
NK = UNITS // 128              # 8 k-chunks of the recurrent contraction
GATES = [1, 0, 2, 3]           # processing order f,i,g,o (orig packing i,f,c,o)
HT_ORDER = (0, 4, 1, 5, 2, 6, 3, 7)   # k-chunk consumption order

_NC_CACHE = {}


def _build(n_warm: int, n_out: int):
    """Build the per-core Bass program. n_out = number of predictions (32)."""
    key = (n_warm, n_out)
    if key in _NC_CACHE:
        return _NC_CACHE[key]

    n_dec = n_out - 1  # LSTM steps in decode phase

    nc = bacc.Bacc("TRN2", target_bir_lowering=False, debug=False,
                   num_devices=N_CORES)
    xt_ext = nc.dram_tensor("xt", [n_warm, F_DIM + 1, B], BF16,
                            kind="ExternalInput")
    wr_ext = nc.dram_tensor("wr", [128, NK * 8 * 512], BF16,
                            kind="ExternalInput")
    wk_ext = nc.dram_tensor("wk", [F_DIM + 1, 8 * 512], BF16,
                            kind="ExternalInput")
    wd_ext = nc.dram_tensor("wd", [128, NK * F_DIM], BF16,
                            kind="ExternalInput")
    bd_ext = nc.dram_tensor("bd", [F_DIM, 1], F32, kind="ExternalInput")
    out_ext = nc.dram_tensor("out", [F_DIM, n_out * B], F32,
                             kind="ExternalOutput")

    with ExitStack() as ctx:
        tc = ctx.enter_context(tile.TileContext(nc))
        wpool = ctx.enter_context(tc.tile_pool(name="w", bufs=1))
        xpool = ctx.enter_context(tc.tile_pool(name="x", bufs=3))
        hTpool = ctx.enter_context(tc.tile_pool(name="hT", bufs=2))
        hpool = ctx.enter_context(tc.tile_pool(name="h", bufs=2))
        cpool = ctx.enter_context(tc.tile_pool(name="c", bufs=2))
        gpool = ctx.enter_context(tc.tile_pool(name="g", bufs=2))
        zpool = ctx.enter_context(tc.tile_pool(name="z", bufs=1, space="PSUM"))
        zspool = ctx.enter_context(tc.tile_pool(name="zs", bufs=2, space="PSUM"))
        tpool = ctx.enter_context(tc.tile_pool(name="tp", bufs=1, space="PSUM"))

        wr_sb = wpool.tile([128, NK * 8 * 512], BF16)
        nc.sync.dma_start(wr_sb[:], wr_ext[:])
        wk_sb = wpool.tile([F_DIM + 1, 8 * 512], BF16)
        nc.sync.dma_start(wk_sb[:], wk_ext[:])
        wd_sb = wpool.tile([128, NK * F_DIM], BF16)
        nc.sync.dma_start(wd_sb[:], wd_ext[:])
        bd_sb = wpool.tile([F_DIM, 1], F32)
        nc.sync.dma_start(bd_sb[:], bd_ext[:])
        identb = wpool.tile([128, 128], BF16)
        make_identity(nc, identb[:])
        preds_sb = wpool.tile([F_DIM, n_out * B], F32)
        xd_sb = wpool.tile([F_DIM + 1, B], BF16)
        nc.vector.memset(xd_sb[F_DIM:F_DIM + 1, :], 1.0)

        state = {"h0": None, "h1": None, "c": None}
        # hT column layout: transpose of h[:, j*128:(j+1)*128] yields unit
        # chunks j (cols 0:64) and j+4 (cols 64:128); store them adjacently
        # so each transpose pair needs ONE contiguous DVE copy.
        HT_POS = {}
        for j in range(4):
            HT_POS[j] = 2 * j
            HT_POS[j + 4] = 2 * j + 1

        def hT_sl(k):
            p = HT_POS[k]
            t = state["hTa"] if p < 4 else state["hTb"]
            return t[:, (p % 4) * B:(p % 4 + 1) * B]

        def transpose_pair(half):
            """h half (bf16, batch-major split layout) -> hT chunks (bf16).
            half 0: chunks 0,4,1,5 from h0; half 1: chunks 2,6,3,7 from h1.
            Each pair gets its own PSUM bank and its own hT SBUF tile."""
            js = (0, 1) if half == 0 else (2, 3)
            tag = "tp0" if half == 0 else "tps"
            tps = tpool.tile([128, 1024], BF16, name=tag, tag=tag)[:, 0:256]
            hT = hTpool.tile([128, 4 * B], BF16,
                             name="hTa" if half == 0 else "hTb",
                             tag="hTa" if half == 0 else "hTb")
            state["hTa" if half == 0 else "hTb"] = hT
            for jj, j in enumerate(js):
                h_half = state["h0"] if j < 2 else state["h1"]
                nc.tensor.transpose(tps[:, jj * 128:(jj + 1) * 128],
                                    h_half[:, (j % 2) * 128:(j % 2 + 1) * 128],
                                    identb[:])
            nc.vector.tensor_copy(hT[:], tps[:])

        def alloc_z():
            """Gate z PSUM tiles, processing order f, i, g, o. f and i are
            full [128,512] banks; g and o are two half-used banks each (a
            shared bank would serialize reads against the bank-mate's
            writes)."""
            zf = zpool.tile([128, 512], F32, name="zf", tag="zf")
            zi = zpool.tile([128, 512], F32, name="zi", tag="zi")
            zg = [zspool.tile([128, 512], F32, name="zg%d" % s, tag="zg")[:, 0:256]
                  for s in (0, 1)]
            zo = [zspool.tile([128, 512], F32, name="zo%d" % s, tag="zo")[:, 0:256]
                  for s in (0, 1)]
            return (zf, zi, zg, zo)

        def runs_of(zs):
            """Six runs: (blk, psum_region, col_lo_in_block, width). f and i
            are single 512-wide runs (one PSUM accumulation group per bank —
            start=True zero-marks the whole 2KB bank row, so a bank must
            never hold two open groups); g and o are 256-wide in their own
            banks. blk = processing-order gate (0=f 1=i 2=g 3=o)."""
            zf, zi, zg, zo = zs
            return (
                (0, zf[:, 0:512], 0, 512),
                (1, zi[:, 0:512], 0, 512),
                (2, zg[0], 0, 256), (2, zg[1], 256, 256),
                (3, zo[0], 0, 256), (3, zo[1], 256, 256),
            )

        def emit_xwk_run(run, x_sb, start, stop):
            """x @ Wk (+b) pair for one run. skip_group_check on the half-1
            opener: CoreSim's zero-region group view is partition-unaware
            and false-positives on the second (partition 64:128) opener of
            a bank; on HW the two halves zero disjoint partition rows."""
            blk, z, lo, w = run
            for half in (0, 1):
                o = (half * 4 + blk) * 512 + lo
                nc.tensor.matmul(z[half * 64:(half + 1) * 64, :],
                                 x_sb[:], wk_sb[:, o:o + w],
                                 start=start, stop=stop,
                                 skip_group_check=(half == 1))

        def emit_hwr_run(run, kis, start_at_first, stop_at_last):
            """h @ Wr chunk-pairs for one run, chunks kis (actual k values)."""
            blk, z, lo, w = run
            for idx, k in enumerate(kis):
                for half in (0, 1):
                    start = start_at_first and idx == 0
                    stop = stop_at_last and idx == len(kis) - 1
                    o = (k * 8 + half * 4 + blk) * 512 + lo
                    nc.tensor.matmul(
                        z[half * 64:(half + 1) * 64, :],
                        hT_sl(k), wr_sb[:, o:o + w],
                        start=start, stop=stop,
                        skip_group_check=(half == 1))

        def pred_block(d):
            """pred_d^T = Wd^T @ h + bd from current hT; returns x_dec tile."""
            # shares the pair-0 transpose bank (released right after copy-a)
            pp = tpool.tile([F_DIM, 512], F32, name="pp", tag="tp0")[:, 0:B]
            for ki, k in enumerate(HT_ORDER):
                nc.tensor.matmul(pp[:], wd_sb[:, k * F_DIM:(k + 1) * F_DIM],
                                 hT_sl(k), start=(ki == 0), stop=(ki == 7))
            # Copies on ScalarE (off the DVE queue); bd is per-partition on
            # pred^T so it folds into the copy as an Identity bias.
            nc.scalar.activation(preds_sb[:, d * B:(d + 1) * B], pp[:],
                                 AF.Identity, bias=bd_sb[:])
            if d < n_out - 1:
                nc.scalar.activation(xd_sb[0:F_DIM, :], pp[:],
                                     AF.Identity, bias=bd_sb[:])
                return xd_sb
            return None

        def gate_chain(zs, first):
            """ACT/DVE emission in engine-FIFO readiness order. fp16 gates
            and cell state (DVE 4x mode), bf16 h (PE operand)."""
            zf, zi, zg, zo = zs
            c_prev = state["c"]
            sig_f, sig_i, tanh_g, sig_o, tanh_c = [], [], [], [], []
            cs, hs, t1s = [None, None], [None, None], [None, None]

            def act(dst_list, src, func, s, tag):
                t = gpool.tile([128, 256], F16, tag="%s%d" % (tag, s),
                               name=tag)
                nc.scalar.activation(t[:], src, func)
                dst_list.append(t)

            if not first:
                # sig_f0, sig_f1 then the two c-mults (DVE) run early
                act(sig_f, zf[:, 0:256], AF.Sigmoid, 0, "sf")
                act(sig_f, zf[:, 256:512], AF.Sigmoid, 1, "sf")
                for s in (0, 1):
                    cs[s] = cpool.tile([128, 256], F16, tag="c%d" % s, name="c")
                    nc.vector.tensor_tensor(cs[s][:], sig_f[s][:],
                                            c_prev[s][:], Alu.mult)
            act(sig_i, zi[:, 0:256], AF.Sigmoid, 0, "si")
            act(sig_i, zi[:, 256:512], AF.Sigmoid, 1, "si")
            for s in (0, 1):
                # tanh_g_s, then DVE t1_s (and c accumulation), then
                # tanh_c_s as soon as c_s is final.
                act(tanh_g, zg[s][:], AF.Tanh, s, "tg")
                if first:
                    cs[s] = cpool.tile([128, 256], F16, tag="c%d" % s, name="c")
                    nc.vector.tensor_tensor(cs[s][:], sig_i[s][:],
                                            tanh_g[s][:], Alu.mult)
                else:
                    t1s[s] = gpool.tile([128, 256], F16, tag="t1_%d" % s,
                                        name="t1")
                    nc.vector.tensor_tensor(t1s[s][:], sig_i[s][:],
                                            tanh_g[s][:], Alu.mult)
                    nc.vector.tensor_tensor(cs[s][:], cs[s][:], t1s[s][:],
                                            Alu.add)
                act(tanh_c, cs[s][:], AF.Tanh, s, "tc")
            for s in (0, 1):
                act(sig_o, zo[s][:], AF.Sigmoid, s, "so")
                h = hpool.tile([128, 256], BF16, tag="h%d" % s, name="h")
                nc.vector.tensor_tensor(h[:], sig_o[s][:], tanh_c[s][:],
                                        Alu.mult)
                hs[s] = h
            state["h0"], state["h1"], state["c"] = hs[0], hs[1], cs

        # ---- warmup ----
        for t in range(n_warm):
            x_sb = xpool.tile([F_DIM + 1, B], BF16)
            nc.sync.dma_start(x_sb[:], xt_ext[t])
            zs = alloc_z()
            runs = runs_of(zs)
            if t == 0:
                for run in runs:
                    emit_xwk_run(run, x_sb, start=True, stop=True)
            else:
                # f/i/g x@Wk runs first (group openers, no hT dependency):
                # PE work covering the previous step's h tail + transposes.
                # The o runs' PSUM buffers are read (sig_o) only at the very
                # end of the previous step, so their x@Wk (whose start=True
                # zero-marks the region) must execute well after the
                # boundary — emit them after the g h@Wr runs.
                for run in runs[:4]:
                    emit_xwk_run(run, x_sb, start=True, stop=False)
                transpose_pair(0)
                transpose_pair(1)
                for run in runs[:4]:
                    emit_hwr_run(run, HT_ORDER, False, True)
                for run in runs[4:]:
                    emit_xwk_run(run, x_sb, start=True, stop=False)
                for run in runs[4:]:
                    emit_hwr_run(run, HT_ORDER, False, True)
            gate_chain(zs, first=(t == 0))

        # ---- decode: h@Wr opens each accumulation group, x@Wk (from the
        # freshly computed pred) closes it at the end of the stream ----
        for d in range(n_dec):
            zs = alloc_z()
            runs = runs_of(zs)
            transpose_pair(0)
            transpose_pair(1)
            emit_hwr_run(runs[0], HT_ORDER, True, False)
            emit_hwr_run(runs[1], HT_ORDER, True, False)
            xd = pred_block(d)
            for run in runs[2:]:
                emit_hwr_run(run, HT_ORDER, True, False)
            for run in runs:
                emit_xwk_run(run, xd, start=False, stop=True)
            gate_chain(zs, first=False)
        transpose_pair(0)
        transpose_pair(1)
        pred_block(n_out - 1)

        nc.sync.dma_start(out_ext[:], preds_sb[:])

    nc.finalize()
    _NC_CACHE[key] = nc
    return nc


def _prep_core_inputs(inputs, Wk, Wr, b, Wd, bd, n_warm, n_out):
    """Host-side reshaping/sharding. Returns list of 8 input dicts."""
    bf = lambda a: np.ascontiguousarray(a).astype(ml_dtypes.bfloat16)
    perm = np.array([g * UNITS + hh * 512 + k
                     for hh in (0, 1) for g in GATES for k in range(512)])
    Wk_aug = np.concatenate([Wk, b[None, :]], 0)[:, perm]        # [65, 4096]
    Wr_p = Wr[:, perm]                                           # [1024, 4096]
    wr_dev = bf(np.stack([Wr_p[k * 128:(k + 1) * 128] for k in range(NK)],
                         1).reshape(128, -1))
    wk_dev = bf(Wk_aug)
    wd_dev = bf(np.stack([Wd[k * 128:(k + 1) * 128] for k in range(NK)],
                         1).reshape(128, -1))
    bd_dev = np.ascontiguousarray(bd[:, None]).astype(np.float32)

    in_maps = []
    for c in range(N_CORES):
        xs = inputs[c * B:(c + 1) * B, :n_warm]                  # [64, T, F]
        xt = xs.transpose(1, 2, 0)                               # [T, F, 64]
        xt_aug = np.concatenate(
            [xt, np.ones((n_warm, 1, B), np.float32)], 1)        # [T, 65, 64]
        in_maps.append({
            "xt": bf(xt_aug), "wr": wr_dev, "wk": wk_dev,
            "wd": wd_dev, "bd": bd_dev,
        })
    return in_maps


def kernel(inputs, Wk, Wr, b, Wd, bd, out_steps):
    inputs = np.asarray(inputs, np.float32)
    Wk = np.asarray(Wk, np.float32)
    Wr = np.asarray(Wr, np.float32)
    b = np.asarray(b, np.float32)
    Wd = np.asarray(Wd, np.float32)
    bd = np.asarray(bd, np.float32)
    n_out = int(out_steps)
    n_warm = inputs.shape[1]

    nc = _build(n_warm, n_out)
    in_maps = _prep_core_inputs(inputs, Wk, Wr, b, Wd, bd, n_warm, n_out)
    res = run_bass_kernel_spmd(nc, in_maps, core_ids=list(range(N_CORES)))

    out = np.empty((B_FULL, n_out, F_DIM), np.float32)
    for c in range(N_CORES):
        o = res.results[c]["out"].reshape(F_DIM, n_out, B)       # [F, t, b]
        out[c * B:(c + 1) * B] = o.transpose(2, 1, 0)
    return out


# revision 18
# speedup vs baseline: 1.3109x; 1.0027x over previous
# Trainium2 Bass kernel for nn_AutoRegressive (LSTM warmup + autoregressive decode).
#
# Problem: B=512, T=128, F=64, UNITS=1024, OUT_STEPS=32.
#   warmup: 128 sequential LSTM steps over inputs, keep final (h, c)
#   decode: pred = h @ Wd + bd, feed pred back as x for 31 more steps
#   output: [B, 32, F]
#
# Strategy: pure 8-way data parallelism on the batch axis (64 rows/core),
# weights replicated, zero cross-core communication. Per step the dominant
# matmul z = x @ Wk + h @ Wr is computed with h^T-stationary matmuls
# (lhsT = h^T[k-chunk] [128, 64]) streaming Wr columns. Because the local
# batch is 64 (< 128 array columns), each matmul pair is column-tiled at
# (0,0)/(0,64) covering the lo/hi unit-halves of a gate concurrently.
#
# Restructure vs the 1.85ms baseline (measured ~1.66ms, ~9.4us/step):
#  - Per warm step the gates are emitted as column runs in completion
#    order f(512), i(512), g0, g1, o0, o1 (256 each), each run = its x@Wk
#    pair (group opener) followed by its full 8-chunk h@Wr k-loop. Gate
#    slices complete progressively through the PE stream so the
#    sigmoid/tanh/cell chain runs concurrently with the matmuls instead
#    of serially at the step tail (was ~3.4us of tail).
#  - PSUM rule learned the hard way (and verified in CoreSim): start=True
#    zero-marks the whole 2KB bank row per written partition, so a bank
#    may hold only ONE open accumulation group at a time. f and i are
#    single 512-wide groups in their banks; g/o pairs live in separate
#    banks. CoreSim's group checker is partition-unaware, so the
#    partition-64:128 twin of each opener sets skip_group_check.
#  - Gate/cell elementwise chain in fp16 (c state, sigmoids, tanh) and
#    bf16 (h): 2-byte dtypes + all-SBUF operands put the DVE in its 4x
#    mode (~510ns -> ~130ns per [128,256] op).
#  - Engines are FIFO: ACT/DVE ops are emitted in exact readiness order
#    (sig_f0..sig_o1 / c-chain) so nothing blocks the h tail.
#  - The next step's f/i/g x@Wk runs are emitted before the transposes
#    and its o x@Wk between the two transpose pairs, giving the PE work
#    while the previous step's h tail completes (the o banks' previous
#    groups are read by then; their x@Wk must not execute near the
#    boundary of the PREVIOUS step, which is also why they are not
#    hoisted further).
#  - Filler matmuls dropped: PE gaps are now well under the ~3.4us HAM
#    re-throttle window.
# Bias b is folded into an augmented ones-row of x / extra row of Wk.
# pred copies run on ScalarE with bd folded in as an Identity bias.
import os
import sys

sys.path.insert(0, "/opt/trn_rl_repo")

import numpy as np
import ml_dtypes

import concourse.bass as bass
import concourse.mybir as mybir
import concourse.tile as tile
from concourse import bacc
from concourse.bass_utils import run_bass_kernel_spmd
from concourse.masks import make_identity
from contextlib import ExitStack

F32, F16, BF16 = mybir.dt.float32, mybir.dt.float16, mybir.dt.bfloat16
AF = mybir.ActivationFunctionType
Alu = mybir.AluOpType

B_FULL, T_FULL, F_DIM, UNITS = 512, 128, 64, 1024
N_CORES = 8
B = B_FULL // N_CORES          # 64 local batch rows
NK = UNITS // 128              # 8 k-chunks of the recurrent contraction
GATES = [1, 0, 2, 3]           # processing order f,i,g,o (orig packing i,f,c,o)
HT_ORDER = (0, 4, 1, 5, 2, 6, 3, 7)   # k-chunk consumption order

_NC_CACHE = {}


def _build(n_warm: int, n_out: int):
    """Build the per-core Bass program. n_out = number of predictions (32)."""
    key = (n_warm, n_out)
    if key in _NC_CACHE:
        return _NC_CACHE[key]

    n_dec = n_out - 1  # LSTM steps in decode phase

    nc = bacc.Bacc("TRN2", target_bir_lowering=False, debug=False,
                   num_devices=N_CORES)
    xt_ext = nc.dram_tensor("xt", [n_warm, F_DIM + 1, B], BF16,
                            kind="ExternalInput")
    wr_ext = nc.dram_tensor("wr", [128, NK * 8 * 512], BF16,
                            kind="ExternalInput")
    wk_ext = nc.dram_tensor("wk", [F_DIM + 1, 8 * 512], BF16,
                            kind="ExternalInput")
    wd_ext = nc.dram_tensor("wd", [128, NK * F_DIM], BF16,
                            kind="ExternalInput")
    bd_ext = nc.dram_tensor("bd", [F_DIM, 1], F32, kind="ExternalInput")
    out_ext = nc.dram_tensor("out", [F_DIM, n_out * B], F32,
                             kind="ExternalOutput")

    with ExitStack() as ctx:
        tc = ctx.enter_context(tile.TileContext(nc))
        wpool = ctx.enter_context(tc.tile_pool(name="w", bufs=1))
        xpool = ctx.enter_context(tc.tile_pool(name="x", bufs=3))
        hTpool = ctx.enter_context(tc.tile_pool(name="hT", bufs=2))
        hpool = ctx.enter_context(tc.tile_pool(name="h", bufs=2))
        cpool = ctx.enter_context(tc.tile_pool(name="c", bufs=2))
        gpool = ctx.enter_context(tc.tile_pool(name="g", bufs=2))
        zpool = ctx.enter_context(tc.tile_pool(name="z", bufs=1, space="PSUM"))
        zspool = ctx.enter_context(tc.tile_pool(name="zs", bufs=2, space="PSUM"))
        tpool = ctx.enter_context(tc.tile_pool(name="tp", bufs=1, space="PSUM"))

        wr_sb = wpool.tile([128, NK * 8 * 512], BF16)
        nc.sync.dma_start(wr_sb[:], wr_ext[:])
        wk_sb = wpool.tile([F_DIM + 1, 8 * 512], BF16)
        nc.sync.dma_start(wk_sb[:], wk_ext[:])
        wd_sb = wpool.tile([128, NK * F_DIM], BF16)
        nc.sync.dma_start(wd_sb[:], wd_ext[:])
        bd_sb = wpool.tile([F_DIM, 1], F32)
        nc.sync.dma_start(bd_sb[:], bd_ext[:])
        identb = wpool.tile([128, 128], BF16)
        make_identity(nc, identb[:])
        preds_sb = wpool.tile([F_DIM, n_out * B], F32)
        xd_sb = wpool.tile([F_DIM + 1, B], BF16)
        nc.vector.memset(xd_sb[F_DIM:F_DIM + 1, :], 1.0)

        state = {"h0": None, "h1": None, "c": None}
        # hT column layout: transpose of h[:, j*128:(j+1)*128] yields unit
        # chunks j (cols 0:64) and j+4 (cols 64:128); store them adjacently
        # so each transpose pair needs ONE contiguous DVE copy.
        HT_POS = {}
        for j in range(4):
            HT_POS[j] = 2 * j
            HT_POS[j + 4] = 2 * j + 1

        def hT_sl(k):
            p = HT_POS[k]
            t = state["hTa"] if p < 4 else state["hTb"]
            return t[:, (p % 4) * B:(p % 4 + 1) * B]

        def transpose_pair(half):
            """h half (bf16, batch-major split layout) -> hT chunks (bf16).
            half 0: chunks 0,4,1,5 from h0; half 1: chunks 2,6,3,7 from h1.
            Each pair gets its own PSUM bank and its own hT SBUF tile."""
            js = (0, 1) if half == 0 else (2, 3)
            tag = "tp0" if half == 0 else "tps"
            tps = tpool.tile([128, 1024], BF16, name=tag, tag=tag)[:, 0:256]
            hT = hTpool.tile([128, 4 * B], BF16,
                             name="hTa" if half == 0 else "hTb",
                             tag="hTa" if half == 0 else "hTb")
            state["hTa" if half == 0 else "hTb"] = hT
            for jj, j in enumerate(js):
                h_half = state["h0"] if j < 2 else state["h1"]
                nc.tensor.transpose(tps[:, jj * 128:(jj + 1) * 128],
                                    h_half[:, (j % 2) * 128:(j % 2 + 1) * 128],
                                    identb[:])
            nc.vector.tensor_copy(hT[:], tps[:])

        def alloc_z():
            """Gate z PSUM tiles, processing order f, i, g, o. f and i are
            full [128,512] banks; g and o are two half-used banks each (a
            shared bank would serialize reads against the bank-mate's
            writes)."""
            zf = zpool.tile([128, 512], F32, name="zf", tag="zf")
            zi = zpool.tile([128, 512], F32, name="zi", tag="zi")
            zg = [zspool.tile([128, 512], F32, name="zg%d" % s, tag="zg")[:, 0:256]
                  for s in (0, 1)]
            zo = [zspool.tile([128, 512], F32, name="zo%d" % s, tag="zo")[:, 0:256]
                  for s in (0, 1)]
            return (zf, zi, zg, zo)

        def runs_of(zs):
            """Six runs: (blk, psum_region, col_lo_in_block, width). f and i
            are single 512-wide runs (one PSUM accumulation group per bank —
            start=True zero-marks the whole 2KB bank row, so a bank must
            never hold two open groups); g and o are 256-wide in their own
            banks. blk = processing-order gate (0=f 1=i 2=g 3=o)."""
            zf, zi, zg, zo = zs
            return (
                (0, zf[:, 0:512], 0, 512),
                (1, zi[:, 0:512], 0, 512),
                (2, zg[0], 0, 256), (2, zg[1], 256, 256),
                (3, zo[0], 0, 256), (3, zo[1], 256, 256),
            )

        def emit_xwk_run(run, x_sb, start, stop):
            """x @ Wk (+b) pair for one run. skip_group_check on the half-1
            opener: CoreSim's zero-region group view is partition-unaware
            and false-positives on the second (partition 64:128) opener of
            a bank; on HW the two halves zero disjoint partition rows."""
            blk, z, lo, w = run
            for half in (0, 1):
                o = (half * 4 + blk) * 512 + lo
                nc.tensor.matmul(z[half * 64:(half + 1) * 64, :],
                                 x_sb[:], wk_sb[:, o:o + w],
                                 start=start, stop=stop,
                                 skip_group_check=(half == 1))

        def emit_hwr_run(run, kis, start_at_first, stop_at_last):
            """h @ Wr chunk-pairs for one run, chunks kis (actual k values)."""
            blk, z, lo, w = run
            for idx, k in enumerate(kis):
                for half in (0, 1):
                    start = start_at_first and idx == 0
                    stop = stop_at_last and idx == len(kis) - 1
                    o = (k * 8 + half * 4 + blk) * 512 + lo
                    nc.tensor.matmul(
                        z[half * 64:(half + 1) * 64, :],
                        hT_sl(k), wr_sb[:, o:o + w],
                        start=start, stop=stop,
                        skip_group_check=(half == 1))

        def pred_block(d):
            """pred_d^T = Wd^T @ h + bd from current hT; returns x_dec tile."""
            # shares the pair-0 transpose bank (released right after copy-a)
            pp = tpool.tile([F_DIM, 512], F32, name="pp", tag="tp0")[:, 0:B]
            for ki, k in enumerate(HT_ORDER):
                nc.tensor.matmul(pp[:], wd_sb[:, k * F_DIM:(k + 1) * F_DIM],
                                 hT_sl(k), start=(ki == 0), stop=(ki == 7))
            # Copies on ScalarE (off the DVE queue); bd is per-partition on
            # pred^T so it folds into the copy as an Identity bias.
            nc.scalar.activation(preds_sb[:, d * B:(d + 1) * B], pp[:],
                                 AF.Identity, bias=bd_sb[:])
            if d < n_out - 1:
                nc.scalar.activation(xd_sb[0:F_DIM, :], pp[:],
                                     AF.Identity, bias=bd_sb[:])
                return xd_sb
            return None

        def gate_chain(zs, first):
            """ACT/DVE emission in engine-FIFO readiness order. fp16 gates
            and cell state (DVE 4x mode), bf16 h (PE operand)."""
            zf, zi, zg, zo = zs
            c_prev = state["c"]
            sig_f, sig_i, tanh_g, sig_o, tanh_c = [], [], [], [], []
            cs, hs, t1s = [None, None], [None, None], [None, None]

            def act(dst_list, src, func, s, tag):
                t = gpool.tile([128, 256], F16, tag="%s%d" % (tag, s),
                               name=tag)
                nc.scalar.activation(t[:], src, func)
                dst_list.append(t)

            if not first:
                # sig_f0, sig_f1 then the two c-mults (DVE) run early
                act(sig_f, zf[:, 0:256], AF.Sigmoid, 0, "sf")
                act(sig_f, zf[:, 256:512], AF.Sigmoid, 1, "sf")
                for s in (0, 1):
                    cs[s] = cpool.tile([128, 256], F16, tag="c%d" % s, name="c")
                    nc.vector.tensor_tensor(cs[s][:], sig_f[s][:],
                                            c_prev[s][:], Alu.mult)
            act(sig_i, zi[:, 0:256], AF.Sigmoid, 0, "si")
            act(sig_i, zi[:, 256:512], AF.Sigmoid, 1, "si")
            for s in (0, 1):
                # tanh_g_s, then DVE t1_s (and c accumulation), then
                # tanh_c_s as soon as c_s is final.
                act(tanh_g, zg[s][:], AF.Tanh, s, "tg")
                if first:
                    cs[s] = cpool.tile([128, 256], F16, tag="c%d" % s, name="c")
                    nc.vector.tensor_tensor(cs[s][:], sig_i[s][:],
                                            tanh_g[s][:], Alu.mult)
                else:
                    t1s[s] = gpool.tile([128, 256], F16, tag="t1_%d" % s,
                                        name="t1")
                    nc.vector.tensor_tensor(t1s[s][:], sig_i[s][:],
                                            tanh_g[s][:], Alu.mult)
                    nc.vector.tensor_tensor(cs[s][:], cs[s][:], t1s[s][:],
                                            Alu.add)
                act(tanh_c, cs[s][:], AF.Tanh, s, "tc")
            for s in (0, 1):
                act(sig_o, zo[s][:], AF.Sigmoid, s, "so")
                h = hpool.tile([128, 256], BF16, tag="h%d" % s, name="h")
                nc.vector.tensor_tensor(h[:], sig_o[s][:], tanh_c[s][:],
                                        Alu.mult)
                hs[s] = h
            state["h0"], state["h1"], state["c"] = hs[0], hs[1], cs

        # ---- warmup ----
        for t in range(n_warm):
            x_sb = xpool.tile([F_DIM + 1, B], BF16)
            nc.sync.dma_start(x_sb[:], xt_ext[t])
            zs = alloc_z()
            runs = runs_of(zs)
            if t == 0:
                for run in runs:
                    emit_xwk_run(run, x_sb, start=True, stop=True)
            else:
                # f/i/g x@Wk runs first (group openers, no hT dependency):
                # PE work covering the previous step's h tail + transposes.
                # The o runs' PSUM buffers are read (sig_o) only at the very
                # end of the previous step, so their x@Wk (whose start=True
                # zero-marks the region) must execute well after the
                # boundary — emit them after the g h@Wr runs.
                for run in runs[:4]:
                    emit_xwk_run(run, x_sb, start=True, stop=False)
                transpose_pair(0)
                # o-gate x@Wk between the transpose pairs: fills the PE
                # wait for h1 (the pair-1 transpose input). Group-legal:
                # zo0/zo1 are their own banks, opened exactly once; the
                # WAR on the previous step's sig_o reads is past by now.
                for run in runs[4:]:
                    emit_xwk_run(run, x_sb, start=True, stop=False)
                transpose_pair(1)
                for run in runs[:4]:
                    emit_hwr_run(run, HT_ORDER, False, True)
                for run in runs[4:]:
                    emit_hwr_run(run, HT_ORDER, False, True)
            gate_chain(zs, first=(t == 0))

        # ---- decode: h@Wr opens each accumulation group, x@Wk (from the
        # freshly computed pred) closes it at the end of the stream ----
        for d in range(n_dec):
            zs = alloc_z()
            runs = runs_of(zs)
            transpose_pair(0)
            transpose_pair(1)
            emit_hwr_run(runs[0], HT_ORDER, True, False)
            emit_hwr_run(runs[1], HT_ORDER, True, False)
            xd = pred_block(d)
            for run in runs[2:]:
                emit_hwr_run(run, HT_ORDER, True, False)
            for run in runs:
                emit_xwk_run(run, xd, start=False, stop=True)
            gate_chain(zs, first=False)
        transpose_pair(0)
        transpose_pair(1)
        pred_block(n_out - 1)

        nc.sync.dma_start(out_ext[:], preds_sb[:])

    nc.finalize()
    _NC_CACHE[key] = nc
    return nc


def _prep_core_inputs(inputs, Wk, Wr, b, Wd, bd, n_warm, n_out):
    """Host-side reshaping/sharding. Returns list of 8 input dicts."""
    bf = lambda a: np.ascontiguousarray(a).astype(ml_dtypes.bfloat16)
    perm = np.array([g * UNITS + hh * 512 + k
                     for hh in (0, 1) for g in GATES for k in range(512)])
    Wk_aug = np.concatenate([Wk, b[None, :]], 0)[:, perm]        # [65, 4096]
    Wr_p = Wr[:, perm]                                           # [1024, 4096]
    wr_dev = bf(np.stack([Wr_p[k * 128:(k + 1) * 128] for k in range(NK)],
                         1).reshape(128, -1))
    wk_dev = bf(Wk_aug)
    wd_dev = bf(np.stack([Wd[k * 128:(k + 1) * 128] for k in range(NK)],
                         1).reshape(128, -1))
    bd_dev = np.ascontiguousarray(bd[:, None]).astype(np.float32)

    in_maps = []
    for c in range(N_CORES):
        xs = inputs[c * B:(c + 1) * B, :n_warm]                  # [64, T, F]
        xt = xs.transpose(1, 2, 0)                               # [T, F, 64]
        xt_aug = np.concatenate(
            [xt, np.ones((n_warm, 1, B), np.float32)], 1)        # [T, 65, 64]
        in_maps.append({
            "xt": bf(xt_aug), "wr": wr_dev, "wk": wk_dev,
            "wd": wd_dev, "bd": bd_dev,
        })
    return in_maps


def kernel(inputs, Wk, Wr, b, Wd, bd, out_steps):
    inputs = np.asarray(inputs, np.float32)
    Wk = np.asarray(Wk, np.float32)
    Wr = np.asarray(Wr, np.float32)
    b = np.asarray(b, np.float32)
    Wd = np.asarray(Wd, np.float32)
    bd = np.asarray(bd, np.float32)
    n_out = int(out_steps)
    n_warm = inputs.shape[1]

    nc = _build(n_warm, n_out)
    in_maps = _prep_core_inputs(inputs, Wk, Wr, b, Wd, bd, n_warm, n_out)
    res = run_bass_kernel_spmd(nc, in_maps, core_ids=list(range(N_CORES)))

    out = np.empty((B_FULL, n_out, F_DIM), np.float32)
    for c in range(N_CORES):
        o = res.results[c]["out"].reshape(F_DIM, n_out, B)       # [F, t, b]
        out[c * B:(c + 1) * B] = o.transpose(2, 1, 0)
    return out


# revision 21
# speedup vs baseline: 1.3931x; 1.0628x over previous
# Trainium2 Bass kernel for nn_AutoRegressive (LSTM warmup + autoregressive decode).
#
# Problem: B=512, T=128, F=64, UNITS=1024, OUT_STEPS=32.
#   warmup: 128 sequential LSTM steps over inputs, keep final (h, c)
#   decode: pred = h @ Wd + bd, feed pred back as x for 31 more steps
#   output: [B, 32, F]
#
# Strategy: pure 8-way data parallelism on the batch axis (64 rows/core),
# weights replicated, zero cross-core communication. Per step the dominant
# matmul z = x @ Wk + h @ Wr is computed with h^T-stationary matmuls
# (lhsT = h^T[k-chunk] [128, 64]) streaming Wr columns. Because the local
# batch is 64 (< 128 array columns), each matmul pair is column-tiled at
# (0,0)/(0,64) covering the lo/hi unit-halves of a gate concurrently.
#
# Restructure vs the 1.85ms baseline (measured ~1.66ms, ~9.4us/step):
#  - Per warm step the gates are emitted as column runs in completion
#    order f(512), i(512), g0, g1, o0, o1 (256 each), each run = its x@Wk
#    pair (group opener) followed by its full 8-chunk h@Wr k-loop. Gate
#    slices complete progressively through the PE stream so the
#    sigmoid/tanh/cell chain runs concurrently with the matmuls instead
#    of serially at the step tail (was ~3.4us of tail).
#  - PSUM rule learned the hard way (and verified in CoreSim): start=True
#    zero-marks the whole 2KB bank row per written partition, so a bank
#    may hold only ONE open accumulation group at a time. f and i are
#    single 512-wide groups in their banks; g/o pairs live in separate
#    banks. CoreSim's group checker is partition-unaware, so the
#    partition-64:128 twin of each opener sets skip_group_check.
#  - Gate/cell elementwise chain in fp16 (c state, sigmoids, tanh) and
#    bf16 (h): 2-byte dtypes + all-SBUF operands put the DVE in its 4x
#    mode (~510ns -> ~130ns per [128,256] op).
#  - Engines are FIFO: ACT/DVE ops are emitted in exact readiness order
#    (sig_f0..sig_o1 / c-chain) so nothing blocks the h tail.
#  - The next step's f/i/g x@Wk runs are emitted before the transposes
#    and its o x@Wk between the two transpose pairs, giving the PE work
#    while the previous step's h tail completes (the o banks' previous
#    groups are read by then; their x@Wk must not execute near the
#    boundary of the PREVIOUS step, which is also why they are not
#    hoisted further).
#  - Filler matmuls dropped: PE gaps are now well under the ~3.4us HAM
#    re-throttle window.
# Bias b is folded into an augmented ones-row of x / extra row of Wk.
# pred copies run on ScalarE with bd folded in as an Identity bias.
import os
import sys

sys.path.insert(0, "/opt/trn_rl_repo")

import numpy as np
import ml_dtypes

import concourse.bass as bass
import concourse.mybir as mybir
import concourse.tile as tile
from concourse import bacc
from concourse.bass_utils import run_bass_kernel_spmd
from concourse.masks import make_identity
from contextlib import ExitStack

F32, F16, BF16 = mybir.dt.float32, mybir.dt.float16, mybir.dt.bfloat16
AF = mybir.ActivationFunctionType
Alu = mybir.AluOpType

B_FULL, T_FULL, F_DIM, UNITS = 512, 128, 64, 1024
N_CORES = 8
B = B_FULL // N_CORES          # 64 local batch rows
NK = UNITS // 128              # 8 k-chunks of the recurrent contraction
GATES = [1, 0, 2, 3]           # processing order f,i,g,o (orig packing i,f,c,o)
HT_ORDER = (0, 4, 1, 5, 2, 6, 3, 7)   # k-chunk consumption order

_NC_CACHE = {}


def _build(n_warm: int, n_out: int):
    """Build the per-core Bass program. n_out = number of predictions (32)."""
    key = (n_warm, n_out)
    if key in _NC_CACHE:
        return _NC_CACHE[key]

    n_dec = n_out - 1  # LSTM steps in decode phase

    nc = bacc.Bacc("TRN2", target_bir_lowering=False, debug=False,
                   num_devices=N_CORES)
    xt_ext = nc.dram_tensor("xt", [n_warm, F_DIM + 1, B], BF16,
                            kind="ExternalInput")
    wr_ext = nc.dram_tensor("wr", [128, NK * 8 * 512], BF16,
                            kind="ExternalInput")
    wk_ext = nc.dram_tensor("wk", [F_DIM + 1, 8 * 512], BF16,
                            kind="ExternalInput")
    wd_ext = nc.dram_tensor("wd", [128, NK * F_DIM], BF16,
                            kind="ExternalInput")
    bd_ext = nc.dram_tensor("bd", [F_DIM, 1], F32, kind="ExternalInput")
    out_ext = nc.dram_tensor("out", [F_DIM, n_out * B], F32,
                             kind="ExternalOutput")

    with ExitStack() as ctx:
        tc = ctx.enter_context(tile.TileContext(nc))
        wpool = ctx.enter_context(tc.tile_pool(name="w", bufs=1))
        xpool = ctx.enter_context(tc.tile_pool(name="x", bufs=3))
        hTpool = ctx.enter_context(tc.tile_pool(name="hT", bufs=2))
        hpool = ctx.enter_context(tc.tile_pool(name="h", bufs=2))
        cpool = ctx.enter_context(tc.tile_pool(name="c", bufs=2))
        gpool = ctx.enter_context(tc.tile_pool(name="g", bufs=2))
        zpool = ctx.enter_context(tc.tile_pool(name="z", bufs=1, space="PSUM"))
        zspool = ctx.enter_context(tc.tile_pool(name="zs", bufs=2, space="PSUM"))
        tpool = ctx.enter_context(tc.tile_pool(name="tp", bufs=1, space="PSUM"))

        wr_sb = wpool.tile([128, NK * 8 * 512], BF16)
        nc.sync.dma_start(wr_sb[:], wr_ext[:])
        wk_sb = wpool.tile([F_DIM + 1, 8 * 512], BF16)
        nc.sync.dma_start(wk_sb[:], wk_ext[:])
        wd_sb = wpool.tile([128, NK * F_DIM], BF16)
        nc.sync.dma_start(wd_sb[:], wd_ext[:])
        bd_sb = wpool.tile([F_DIM, 1], F32)
        nc.sync.dma_start(bd_sb[:], bd_ext[:])
        identb = wpool.tile([128, 128], BF16)
        make_identity(nc, identb[:])
        preds_sb = wpool.tile([F_DIM, n_out * B], F32)
        xd_sb = wpool.tile([F_DIM + 1, B], BF16)
        nc.vector.memset(xd_sb[F_DIM:F_DIM + 1, :], 1.0)

        state = {"h0": None, "h1": None, "c": None}
        # hT column layout: transpose of h[:, j*128:(j+1)*128] yields unit
        # chunks j (cols 0:64) and j+4 (cols 64:128); store them adjacently
        # so each transpose pair needs ONE contiguous DVE copy.
        HT_POS = {}
        for j in range(4):
            HT_POS[j] = 2 * j
            HT_POS[j + 4] = 2 * j + 1

        def hT_sl(k):
            p = HT_POS[k]
            t = state["hTa"] if p < 4 else state["hTb"]
            return t[:, (p % 4) * B:(p % 4 + 1) * B]

        def transpose_pair(half):
            """h half (bf16, batch-major split layout) -> hT chunks (bf16).
            half 0: chunks 0,4,1,5 from h0; half 1: chunks 2,6,3,7 from h1.
            Each pair gets its own PSUM bank and its own hT SBUF tile."""
            js = (0, 1) if half == 0 else (2, 3)
            tag = "tp0" if half == 0 else "tps"
            tps = tpool.tile([128, 1024], BF16, name=tag, tag=tag)[:, 0:256]
            hT = hTpool.tile([128, 4 * B], BF16,
                             name="hTa" if half == 0 else "hTb",
                             tag="hTa" if half == 0 else "hTb")
            state["hTa" if half == 0 else "hTb"] = hT
            for jj, j in enumerate(js):
                h_half = state["h0"] if j < 2 else state["h1"]
                nc.tensor.transpose(tps[:, jj * 128:(jj + 1) * 128],
                                    h_half[:, (j % 2) * 128:(j % 2 + 1) * 128],
                                    identb[:])
            nc.vector.tensor_copy(hT[:], tps[:])

        def alloc_z():
            """Gate z PSUM tiles, processing order f, i, g, o. f and i are
            full [128,512] banks; g and o are two half-used banks each (a
            shared bank would serialize reads against the bank-mate's
            writes)."""
            zf = zpool.tile([128, 512], F32, name="zf", tag="zf")
            zi = zpool.tile([128, 512], F32, name="zi", tag="zi")
            zg = [zspool.tile([128, 512], F32, name="zg%d" % s, tag="zg")[:, 0:256]
                  for s in (0, 1)]
            zo = [zspool.tile([128, 512], F32, name="zo%d" % s, tag="zo")[:, 0:256]
                  for s in (0, 1)]
            return (zf, zi, zg, zo)

        def runs_of(zs):
            """Six runs: (blk, psum_region, col_lo_in_block, width). f and i
            are single 512-wide runs (one PSUM accumulation group per bank —
            start=True zero-marks the whole 2KB bank row, so a bank must
            never hold two open groups); g and o are 256-wide in their own
            banks. blk = processing-order gate (0=f 1=i 2=g 3=o)."""
            zf, zi, zg, zo = zs
            return (
                (0, zf[:, 0:512], 0, 512),
                (1, zi[:, 0:512], 0, 512),
                (2, zg[0], 0, 256), (2, zg[1], 256, 256),
                (3, zo[0], 0, 256), (3, zo[1], 256, 256),
            )

        def emit_xwk_run(run, x_sb, start, stop):
            """x @ Wk (+b) pair for one run. skip_group_check on the half-1
            opener: CoreSim's zero-region group view is partition-unaware
            and false-positives on the second (partition 64:128) opener of
            a bank; on HW the two halves zero disjoint partition rows."""
            blk, z, lo, w = run
            for half in (0, 1):
                o = (half * 4 + blk) * 512 + lo
                nc.tensor.matmul(z[half * 64:(half + 1) * 64, :],
                                 x_sb[:], wk_sb[:, o:o + w],
                                 start=start, stop=stop,
                                 skip_group_check=(half == 1))

        def emit_hwr_run(run, kis, start_at_first, stop_at_last):
            """h @ Wr chunk-pairs for one run, chunks kis (actual k values)."""
            blk, z, lo, w = run
            for idx, k in enumerate(kis):
                for half in (0, 1):
                    start = start_at_first and idx == 0
                    stop = stop_at_last and idx == len(kis) - 1
                    o = (k * 8 + half * 4 + blk) * 512 + lo
                    nc.tensor.matmul(
                        z[half * 64:(half + 1) * 64, :],
                        hT_sl(k), wr_sb[:, o:o + w],
                        start=start, stop=stop,
                        skip_group_check=(half == 1))

        def pred_block(d):
            """pred_d^T = Wd^T @ h + bd from current hT; returns x_dec tile."""
            # shares the pair-0 transpose bank (released right after copy-a)
            pp = tpool.tile([F_DIM, 512], F32, name="pp", tag="tp0")[:, 0:B]
            for ki, k in enumerate(HT_ORDER):
                nc.tensor.matmul(pp[:], wd_sb[:, k * F_DIM:(k + 1) * F_DIM],
                                 hT_sl(k), start=(ki == 0), stop=(ki == 7))
            # Copies on ScalarE (off the DVE queue); bd is per-partition on
            # pred^T so it folds into the copy as an Identity bias. The xd
            # copy goes first: it gates the decode x@Wk matmuls.
            ret = None
            if d < n_out - 1:
                nc.scalar.activation(xd_sb[0:F_DIM, :], pp[:],
                                     AF.Identity, bias=bd_sb[:])
                ret = xd_sb
            nc.scalar.activation(preds_sb[:, d * B:(d + 1) * B], pp[:],
                                 AF.Identity, bias=bd_sb[:])
            return ret

        def gate_chain(zs, first):
            """ACT/DVE emission in engine-FIFO readiness order. fp16 gates
            and cell state (DVE 4x mode), bf16 h (PE operand)."""
            zf, zi, zg, zo = zs
            c_prev = state["c"]
            sig_f, sig_i, tanh_g, sig_o, tanh_c = [], [], [], [], []
            cs, hs, t1s = [None, None], [None, None], [None, None]

            def act(dst_list, src, func, s, tag):
                t = gpool.tile([128, 256], F16, tag="%s%d" % (tag, s),
                               name=tag)
                nc.scalar.activation(t[:], src, func)
                dst_list.append(t)

            if not first:
                # sig_f0, sig_f1 then the two c-mults (DVE) run early
                act(sig_f, zf[:, 0:256], AF.Sigmoid, 0, "sf")
                act(sig_f, zf[:, 256:512], AF.Sigmoid, 1, "sf")
                for s in (0, 1):
                    cs[s] = cpool.tile([128, 256], F16, tag="c%d" % s, name="c")
                    nc.vector.tensor_tensor(cs[s][:], sig_f[s][:],
                                            c_prev[s][:], Alu.mult)
            act(sig_i, zi[:, 0:256], AF.Sigmoid, 0, "si")
            act(sig_i, zi[:, 256:512], AF.Sigmoid, 1, "si")

            def c_update(s):
                if first:
                    cs[s] = cpool.tile([128, 256], F16, tag="c%d" % s, name="c")
                    nc.vector.tensor_tensor(cs[s][:], sig_i[s][:],
                                            tanh_g[s][:], Alu.mult)
                else:
                    t1s[s] = gpool.tile([128, 256], F16, tag="t1_%d" % s,
                                        name="t1")
                    nc.vector.tensor_tensor(t1s[s][:], sig_i[s][:],
                                            tanh_g[s][:], Alu.mult)
                    nc.vector.tensor_tensor(cs[s][:], cs[s][:], t1s[s][:],
                                            Alu.add)

            # ACT FIFO order matters: sig_o0 goes BEFORE tanh_c1 (its zo0
            # input is ready ~1us earlier) so h0 - and with it the pair-0
            # transpose + hT copy of the next step - fires as early as
            # possible instead of behind the slice-1 c chain.
            act(tanh_g, zg[0][:], AF.Tanh, 0, "tg")
            c_update(0)
            act(tanh_c, cs[0][:], AF.Tanh, 0, "tc")
            act(tanh_g, zg[1][:], AF.Tanh, 1, "tg")
            c_update(1)
            act(sig_o, zo[0][:], AF.Sigmoid, 0, "so")
            act(tanh_c, cs[1][:], AF.Tanh, 1, "tc")
            act(sig_o, zo[1][:], AF.Sigmoid, 1, "so")
            for s in (0, 1):
                h = hpool.tile([128, 256], BF16, tag="h%d" % s, name="h")
                nc.vector.tensor_tensor(h[:], sig_o[s][:], tanh_c[s][:],
                                        Alu.mult)
                hs[s] = h
            state["h0"], state["h1"], state["c"] = hs[0], hs[1], cs

        # ---- warmup ----
        for t in range(n_warm):
            x_sb = xpool.tile([F_DIM + 1, B], BF16)
            nc.sync.dma_start(x_sb[:], xt_ext[t])
            zs = alloc_z()
            runs = runs_of(zs)
            if t == 0:
                for run in runs:
                    emit_xwk_run(run, x_sb, start=True, stop=True)
            else:
                # f/i/g x@Wk runs first (group openers, no hT dependency):
                # PE work covering the previous step's h tail + transposes.
                # The o runs' PSUM buffers are read (sig_o) only at the very
                # end of the previous step, so their x@Wk (whose start=True
                # zero-marks the region) must execute well after the
                # boundary — emit them after the g h@Wr runs.
                for run in runs[:4]:
                    emit_xwk_run(run, x_sb, start=True, stop=False)
                transpose_pair(0)
                # o-gate x@Wk between the transpose pairs: fills the PE
                # wait for h1 (the pair-1 transpose input). Group-legal:
                # zo0/zo1 are their own banks, opened exactly once; the
                # WAR on the previous step's sig_o reads is past by now.
                for run in runs[4:]:
                    emit_xwk_run(run, x_sb, start=True, stop=False)
                transpose_pair(1)
                for run in runs[:4]:
                    emit_hwr_run(run, HT_ORDER, False, True)
                for run in runs[4:]:
                    emit_hwr_run(run, HT_ORDER, False, True)
            gate_chain(zs, first=(t == 0))

        # ---- decode: pred (and thus xd) is computed right after the
        # transposes; then per gate [h@Wr opens the group; x@Wk closes it]
        # so the gates complete progressively through the stream exactly
        # like in warmup (closing every group at stream end would serialize
        # the whole activation chain after it: +4.3us/step measured) ----
        for d in range(n_dec):
            zs = alloc_z()
            runs = runs_of(zs)
            transpose_pair(0)
            transpose_pair(1)
            xd = pred_block(d)
            for run in runs:
                emit_hwr_run(run, HT_ORDER, True, False)
                emit_xwk_run(run, xd, start=False, stop=True)
            gate_chain(zs, first=False)
        transpose_pair(0)
        transpose_pair(1)
        pred_block(n_out - 1)

        nc.sync.dma_start(out_ext[:], preds_sb[:])

    nc.finalize()
    _NC_CACHE[key] = nc
    return nc


def _prep_core_inputs(inputs, Wk, Wr, b, Wd, bd, n_warm, n_out):
    """Host-side reshaping/sharding. Returns list of 8 input dicts."""
    bf = lambda a: np.ascontiguousarray(a).astype(ml_dtypes.bfloat16)
    perm = np.array([g * UNITS + hh * 512 + k
                     for hh in (0, 1) for g in GATES for k in range(512)])
    Wk_aug = np.concatenate([Wk, b[None, :]], 0)[:, perm]        # [65, 4096]
    Wr_p = Wr[:, perm]                                           # [1024, 4096]
    wr_dev = bf(np.stack([Wr_p[k * 128:(k + 1) * 128] for k in range(NK)],
                         1).reshape(128, -1))
    wk_dev = bf(Wk_aug)
    wd_dev = bf(np.stack([Wd[k * 128:(k + 1) * 128] for k in range(NK)],
                         1).reshape(128, -1))
    bd_dev = np.ascontiguousarray(bd[:, None]).astype(np.float32)

    in_maps = []
    for c in range(N_CORES):
        xs = inputs[c * B:(c + 1) * B, :n_warm]                  # [64, T, F]
        xt = xs.transpose(1, 2, 0)                               # [T, F, 64]
        xt_aug = np.concatenate(
            [xt, np.ones((n_warm, 1, B), np.float32)], 1)        # [T, 65, 64]
        in_maps.append({
            "xt": bf(xt_aug), "wr": wr_dev, "wk": wk_dev,
            "wd": wd_dev, "bd": bd_dev,
        })
    return in_maps


def kernel(inputs, Wk, Wr, b, Wd, bd, out_steps):
    inputs = np.asarray(inputs, np.float32)
    Wk = np.asarray(Wk, np.float32)
    Wr = np.asarray(Wr, np.float32)
    b = np.asarray(b, np.float32)
    Wd = np.asarray(Wd, np.float32)
    bd = np.asarray(bd, np.float32)
    n_out = int(out_steps)
    n_warm = inputs.shape[1]

    nc = _build(n_warm, n_out)
    in_maps = _prep_core_inputs(inputs, Wk, Wr, b, Wd, bd, n_warm, n_out)
    res = run_bass_kernel_spmd(nc, in_maps, core_ids=list(range(N_CORES)))

    out = np.empty((B_FULL, n_out, F_DIM), np.float32)
    for c in range(N_CORES):
        o = res.results[c]["out"].reshape(F_DIM, n_out, B)       # [F, t, b]
        out[c * B:(c + 1) * B] = o.transpose(2, 1, 0)
    return out


# revision 24
# speedup vs baseline: 1.4361x; 1.0309x over previous
# Trainium2 Bass kernel for nn_AutoRegressive (LSTM warmup + autoregressive decode).
#
# Problem: B=512, T=128, F=64, UNITS=1024, OUT_STEPS=32.
#   warmup: 128 sequential LSTM steps over inputs, keep final (h, c)
#   decode: pred = h @ Wd + bd, feed pred back as x for 31 more steps
#   output: [B, 32, F]
#
# Strategy: pure 8-way data parallelism on the batch axis (64 rows/core),
# weights replicated, zero cross-core communication. Per step the dominant
# matmul z = x @ Wk + h @ Wr is computed with h^T-stationary matmuls
# (lhsT = h^T[k-chunk] [128, 64]) streaming Wr columns. Because the local
# batch is 64 (< 128 array columns), each matmul pair is column-tiled at
# (0,0)/(0,64) covering the lo/hi unit-halves of a gate concurrently.
#
# Restructure vs the 1.85ms baseline (measured ~1.66ms, ~9.4us/step):
#  - Per warm step the gates are emitted as column runs in completion
#    order f(512), i(512), g0, g1, o0, o1 (256 each), each run = its x@Wk
#    pair (group opener) followed by its full 8-chunk h@Wr k-loop. Gate
#    slices complete progressively through the PE stream so the
#    sigmoid/tanh/cell chain runs concurrently with the matmuls instead
#    of serially at the step tail (was ~3.4us of tail).
#  - PSUM rule learned the hard way (and verified in CoreSim): start=True
#    zero-marks the whole 2KB bank row per written partition, so a bank
#    may hold only ONE open accumulation group at a time. f and i are
#    single 512-wide groups in their banks; g/o pairs live in separate
#    banks. CoreSim's group checker is partition-unaware, so the
#    partition-64:128 twin of each opener sets skip_group_check.
#  - Gate/cell elementwise chain in fp16 (c state, sigmoids, tanh) and
#    bf16 (h): 2-byte dtypes + all-SBUF operands put the DVE in its 4x
#    mode (~510ns -> ~130ns per [128,256] op).
#  - Engines are FIFO: ACT/DVE ops are emitted in exact readiness order
#    (sig_f0..sig_o1 / c-chain) so nothing blocks the h tail.
#  - The next step's f/i/g x@Wk runs are emitted before the transposes
#    and its o x@Wk between the two transpose pairs, giving the PE work
#    while the previous step's h tail completes (the o banks' previous
#    groups are read by then; their x@Wk must not execute near the
#    boundary of the PREVIOUS step, which is also why they are not
#    hoisted further).
#  - Filler matmuls dropped: PE gaps are now well under the ~3.4us HAM
#    re-throttle window.
# Bias b is folded into an augmented ones-row of x / extra row of Wk.
# pred copies run on ScalarE with bd folded in as an Identity bias.
import os
import sys

sys.path.insert(0, "/opt/trn_rl_repo")

import numpy as np
import ml_dtypes

import concourse.bass as bass
import concourse.mybir as mybir
import concourse.tile as tile
from concourse import bacc
from concourse.bass_utils import run_bass_kernel_spmd
from concourse.masks import make_identity
from contextlib import ExitStack

F32, F16, BF16 = mybir.dt.float32, mybir.dt.float16, mybir.dt.bfloat16
AF = mybir.ActivationFunctionType
Alu = mybir.AluOpType

B_FULL, T_FULL, F_DIM, UNITS = 512, 128, 64, 1024
N_CORES = 8
B = B_FULL // N_CORES          # 64 local batch rows
NK = UNITS // 128              # 8 k-chunks of the recurrent contraction
GATES = [1, 0, 2, 3]           # processing order f,i,g,o (orig packing i,f,c,o)
HT_ORDER = (0, 4, 1, 5, 2, 6, 3, 7)   # k-chunk consumption order

_NC_CACHE = {}


def _build(n_warm: int, n_out: int):
    """Build the per-core Bass program. n_out = number of predictions (32)."""
    key = (n_warm, n_out)
    if key in _NC_CACHE:
        return _NC_CACHE[key]

    n_dec = n_out - 1  # LSTM steps in decode phase

    nc = bacc.Bacc("TRN2", target_bir_lowering=False, debug=False,
                   num_devices=N_CORES)
    xt_ext = nc.dram_tensor("xt", [n_warm, F_DIM + 1, B], BF16,
                            kind="ExternalInput")
    wr_ext = nc.dram_tensor("wr", [128, NK * 8 * 512], BF16,
                            kind="ExternalInput")
    wk_ext = nc.dram_tensor("wk", [F_DIM + 1, 8 * 512], BF16,
                            kind="ExternalInput")
    wd_ext = nc.dram_tensor("wd", [128, NK * F_DIM], BF16,
                            kind="ExternalInput")
    bd_ext = nc.dram_tensor("bd", [F_DIM, 1], F32, kind="ExternalInput")
    out_ext = nc.dram_tensor("out", [F_DIM, n_out * B], F32,
                             kind="ExternalOutput")

    with ExitStack() as ctx:
        tc = ctx.enter_context(tile.TileContext(nc))
        wpool = ctx.enter_context(tc.tile_pool(name="w", bufs=1))
        xpool = ctx.enter_context(tc.tile_pool(name="x", bufs=3))
        hTpool = ctx.enter_context(tc.tile_pool(name="hT", bufs=2))
        hpool = ctx.enter_context(tc.tile_pool(name="h", bufs=2))
        cpool = ctx.enter_context(tc.tile_pool(name="c", bufs=2))
        gpool = ctx.enter_context(tc.tile_pool(name="g", bufs=2))
        zpool = ctx.enter_context(tc.tile_pool(name="z", bufs=1, space="PSUM"))
        zspool = ctx.enter_context(tc.tile_pool(name="zs", bufs=2, space="PSUM"))
        tpool = ctx.enter_context(tc.tile_pool(name="tp", bufs=1, space="PSUM"))

        wr_sb = wpool.tile([128, NK * 8 * 512], BF16)
        nc.sync.dma_start(wr_sb[:], wr_ext[:])
        wk_sb = wpool.tile([F_DIM + 1, 8 * 512], BF16)
        nc.sync.dma_start(wk_sb[:], wk_ext[:])
        wd_sb = wpool.tile([128, NK * F_DIM], BF16)
        nc.sync.dma_start(wd_sb[:], wd_ext[:])
        bd_sb = wpool.tile([F_DIM, 1], F32)
        nc.sync.dma_start(bd_sb[:], bd_ext[:])
        identb = wpool.tile([128, 128], BF16)
        make_identity(nc, identb[:])
        preds_sb = wpool.tile([F_DIM, n_out * B], F32)
        xd_sb = wpool.tile([F_DIM + 1, B], BF16)
        nc.vector.memset(xd_sb[F_DIM:F_DIM + 1, :], 1.0)

        state = {"h0": None, "h1": None, "c": None}
        # hT column layout: transpose of h[:, j*128:(j+1)*128] yields unit
        # chunks j (cols 0:64) and j+4 (cols 64:128); store them adjacently
        # so each transpose pair needs ONE contiguous DVE copy.
        HT_POS = {}
        for j in range(4):
            HT_POS[j] = 2 * j
            HT_POS[j + 4] = 2 * j + 1

        def hT_sl(k):
            p = HT_POS[k]
            t = state["hTa"] if p < 4 else state["hTb"]
            return t[:, (p % 4) * B:(p % 4 + 1) * B]

        def transpose_pair(half):
            """h half (bf16, batch-major split layout) -> hT chunks (bf16).
            half 0: chunks 0,4,1,5 from h0; half 1: chunks 2,6,3,7 from h1.
            Each pair gets its own PSUM bank and its own hT SBUF tile."""
            js = (0, 1) if half == 0 else (2, 3)
            tag = "tp0" if half == 0 else "tps"
            tps = tpool.tile([128, 1024], BF16, name=tag, tag=tag)[:, 0:256]
            hT = hTpool.tile([128, 4 * B], BF16,
                             name="hTa" if half == 0 else "hTb",
                             tag="hTa" if half == 0 else "hTb")
            state["hTa" if half == 0 else "hTb"] = hT
            for jj, j in enumerate(js):
                h_half = state["h0"] if j < 2 else state["h1"]
                nc.tensor.transpose(tps[:, jj * 128:(jj + 1) * 128],
                                    h_half[:, (j % 2) * 128:(j % 2 + 1) * 128],
                                    identb[:])
            nc.vector.tensor_copy(hT[:], tps[:])

        def alloc_z():
            """Gate z PSUM tiles, processing order f, i, g, o. f and i are
            full [128,512] banks; g and o are two half-used banks each (a
            shared bank would serialize reads against the bank-mate's
            writes)."""
            zf = zpool.tile([128, 512], F32, name="zf", tag="zf")
            zi = zpool.tile([128, 512], F32, name="zi", tag="zi")
            zg = [zspool.tile([128, 512], F32, name="zg%d" % s, tag="zg")[:, 0:256]
                  for s in (0, 1)]
            zo = [zspool.tile([128, 512], F32, name="zo%d" % s, tag="zo")[:, 0:256]
                  for s in (0, 1)]
            return (zf, zi, zg, zo)

        def runs_of(zs):
            """Six runs: (blk, psum_region, col_lo_in_block, width). f and i
            are single 512-wide runs (one PSUM accumulation group per bank —
            start=True zero-marks the whole 2KB bank row, so a bank must
            never hold two open groups); g and o are 256-wide in their own
            banks. blk = processing-order gate (0=f 1=i 2=g 3=o)."""
            zf, zi, zg, zo = zs
            return (
                (0, zf[:, 0:512], 0, 512),
                (1, zi[:, 0:512], 0, 512),
                (2, zg[0], 0, 256), (2, zg[1], 256, 256),
                (3, zo[0], 0, 256), (3, zo[1], 256, 256),
            )

        def emit_xwk_run(run, x_sb, start, stop):
            """x @ Wk (+b) pair for one run. skip_group_check on the half-1
            opener: CoreSim's zero-region group view is partition-unaware
            and false-positives on the second (partition 64:128) opener of
            a bank; on HW the two halves zero disjoint partition rows."""
            blk, z, lo, w = run
            for half in (0, 1):
                o = (half * 4 + blk) * 512 + lo
                nc.tensor.matmul(z[half * 64:(half + 1) * 64, :],
                                 x_sb[:], wk_sb[:, o:o + w],
                                 start=start, stop=stop,
                                 skip_group_check=(half == 1))

        def emit_hwr_run(run, kis, start_at_first, stop_at_last):
            """h @ Wr chunk-pairs for one run, chunks kis (actual k values)."""
            blk, z, lo, w = run
            for idx, k in enumerate(kis):
                for half in (0, 1):
                    start = start_at_first and idx == 0
                    stop = stop_at_last and idx == len(kis) - 1
                    o = (k * 8 + half * 4 + blk) * 512 + lo
                    nc.tensor.matmul(
                        z[half * 64:(half + 1) * 64, :],
                        hT_sl(k), wr_sb[:, o:o + w],
                        start=start, stop=stop,
                        skip_group_check=(half == 1))

        def pred_block(d):
            """pred_d^T = Wd^T @ h + bd from current hT; returns x_dec tile."""
            # shares the pair-0 transpose bank (released right after copy-a)
            pp = tpool.tile([F_DIM, 512], F32, name="pp", tag="tp0")[:, 0:B]
            for ki, k in enumerate(HT_ORDER):
                nc.tensor.matmul(pp[:], wd_sb[:, k * F_DIM:(k + 1) * F_DIM],
                                 hT_sl(k), start=(ki == 0), stop=(ki == 7))
            # Copies on ScalarE (off the DVE queue); bd is per-partition on
            # pred^T so it folds into the copy as an Identity bias. The xd
            # copy goes first: it gates the decode x@Wk matmuls.
            ret = None
            if d < n_out - 1:
                nc.scalar.activation(xd_sb[0:F_DIM, :], pp[:],
                                     AF.Identity, bias=bd_sb[:])
                ret = xd_sb
            nc.scalar.activation(preds_sb[:, d * B:(d + 1) * B], pp[:],
                                 AF.Identity, bias=bd_sb[:])
            return ret

        def gate_chain(zs, first):
            """ACT/DVE emission in engine-FIFO readiness order. fp16 gates
            and cell state (DVE 4x mode), bf16 h (PE operand)."""
            zf, zi, zg, zo = zs
            c_prev = state["c"]
            sig_f, sig_i, tanh_g, sig_o, tanh_c = [], [], [], [], []
            cs, hs, t1s = [None, None], [None, None], [None, None]

            def act(dst_list, src, func, s, tag):
                t = gpool.tile([128, 256], F16, tag="%s%d" % (tag, s),
                               name=tag)
                nc.scalar.activation(t[:], src, func)
                dst_list.append(t)

            if not first:
                # sig_f0, sig_f1 then the two c-mults (DVE) run early
                act(sig_f, zf[:, 0:256], AF.Sigmoid, 0, "sf")
                act(sig_f, zf[:, 256:512], AF.Sigmoid, 1, "sf")
                for s in (0, 1):
                    cs[s] = cpool.tile([128, 256], F16, tag="c%d" % s, name="c")
                    nc.vector.tensor_tensor(cs[s][:], sig_f[s][:],
                                            c_prev[s][:], Alu.mult)
            act(sig_i, zi[:, 0:256], AF.Sigmoid, 0, "si")
            act(sig_i, zi[:, 256:512], AF.Sigmoid, 1, "si")

            def c_update(s):
                if first:
                    cs[s] = cpool.tile([128, 256], F16, tag="c%d" % s, name="c")
                    nc.vector.tensor_tensor(cs[s][:], sig_i[s][:],
                                            tanh_g[s][:], Alu.mult)
                else:
                    t1s[s] = gpool.tile([128, 256], F16, tag="t1_%d" % s,
                                        name="t1")
                    nc.vector.tensor_tensor(t1s[s][:], sig_i[s][:],
                                            tanh_g[s][:], Alu.mult)
                    nc.vector.tensor_tensor(cs[s][:], cs[s][:], t1s[s][:],
                                            Alu.add)

            # ACT/DVE emission follows input-readiness order for the
            # f,i,g0,o0,g1,o1 h@Wr run order: the slice-0 chain (tanh_g0 ->
            # c0 -> tanh_c0 -> sig_o0 -> h0) completes mid-stream so the
            # pair-0 transpose + hT copy of the next step fire early, and
            # h1 lands just before the pair-1 transpose needs it.
            def slice_tail(s):
                act(tanh_g, zg[s][:], AF.Tanh, s, "tg")
                c_update(s)
                act(tanh_c, cs[s][:], AF.Tanh, s, "tc")
                act(sig_o, zo[s][:], AF.Sigmoid, s, "so")
                h = hpool.tile([128, 256], BF16, tag="h%d" % s, name="h")
                nc.vector.tensor_tensor(h[:], sig_o[s][:], tanh_c[s][:],
                                        Alu.mult)
                hs[s] = h

            slice_tail(0)
            slice_tail(1)
            state["h0"], state["h1"], state["c"] = hs[0], hs[1], cs

        # ---- warmup ----
        for t in range(n_warm):
            x_sb = xpool.tile([F_DIM + 1, B], BF16)
            nc.sync.dma_start(x_sb[:], xt_ext[t])
            zs = alloc_z()
            runs = runs_of(zs)
            if t == 0:
                for run in runs:
                    emit_xwk_run(run, x_sb, start=True, stop=True)
            else:
                # f/i/g x@Wk runs first (group openers, no hT dependency):
                # PE work covering the previous step's h tail + transposes.
                # The o runs' PSUM buffers are read (sig_o) only at the very
                # end of the previous step, so their x@Wk (whose start=True
                # zero-marks the region) must execute well after the
                # boundary — emit them after the g h@Wr runs.
                for run in runs[:4]:
                    emit_xwk_run(run, x_sb, start=True, stop=False)
                transpose_pair(0)
                # o-gate x@Wk between the transpose pairs: fills the PE
                # wait for h1 (the pair-1 transpose input). Group-legal:
                # zo0/zo1 are their own banks, opened exactly once; the
                # WAR on the previous step's sig_o reads is past by now.
                for run in runs[4:]:
                    emit_xwk_run(run, x_sb, start=True, stop=False)
                transpose_pair(1)
                # h@Wr run order f,i,g0,o0,g1,o1: zo0 closes before zg1 so
                # sig_o0/h0 complete mid-stream (the scheduler orders the
                # ACT queue by readiness; with o0 last the slice-0 h sat
                # behind the whole slice-1 c chain).
                for ri in (0, 1, 2, 4, 3, 5):
                    emit_hwr_run(runs[ri], HT_ORDER, False, True)
            gate_chain(zs, first=(t == 0))

        # ---- decode: pred (and thus xd) is computed right after the
        # transposes; then per gate [h@Wr opens the group; x@Wk closes it]
        # so the gates complete progressively through the stream exactly
        # like in warmup (closing every group at stream end would serialize
        # the whole activation chain after it: +4.3us/step measured) ----
        for d in range(n_dec):
            zs = alloc_z()
            runs = runs_of(zs)
            transpose_pair(0)
            transpose_pair(1)
            xd = pred_block(d)
            for ri in (0, 1, 2, 4, 3, 5):
                emit_hwr_run(runs[ri], HT_ORDER, True, False)
                emit_xwk_run(runs[ri], xd, start=False, stop=True)
            gate_chain(zs, first=False)
        transpose_pair(0)
        transpose_pair(1)
        pred_block(n_out - 1)

        nc.sync.dma_start(out_ext[:], preds_sb[:])

    nc.finalize()
    _NC_CACHE[key] = nc
    return nc


def _prep_core_inputs(inputs, Wk, Wr, b, Wd, bd, n_warm, n_out):
    """Host-side reshaping/sharding. Returns list of 8 input dicts."""
    bf = lambda a: np.ascontiguousarray(a).astype(ml_dtypes.bfloat16)
    perm = np.array([g * UNITS + hh * 512 + k
                     for hh in (0, 1) for g in GATES for k in range(512)])
    Wk_aug = np.concatenate([Wk, b[None, :]], 0)[:, perm]        # [65, 4096]
    Wr_p = Wr[:, perm]                                           # [1024, 4096]
    wr_dev = bf(np.stack([Wr_p[k * 128:(k + 1) * 128] for k in range(NK)],
                         1).reshape(128, -1))
    wk_dev = bf(Wk_aug)
    wd_dev = bf(np.stack([Wd[k * 128:(k + 1) * 128] for k in range(NK)],
                         1).reshape(128, -1))
    bd_dev = np.ascontiguousarray(bd[:, None]).astype(np.float32)

    in_maps = []
    for c in range(N_CORES):
        xs = inputs[c * B:(c + 1) * B, :n_warm]                  # [64, T, F]
        xt = xs.transpose(1, 2, 0)                               # [T, F, 64]
        xt_aug = np.concatenate(
            [xt, np.ones((n_warm, 1, B), np.float32)], 1)        # [T, 65, 64]
        in_maps.append({
            "xt": bf(xt_aug), "wr": wr_dev, "wk": wk_dev,
            "wd": wd_dev, "bd": bd_dev,
        })
    return in_maps


def kernel(inputs, Wk, Wr, b, Wd, bd, out_steps):
    inputs = np.asarray(inputs, np.float32)
    Wk = np.asarray(Wk, np.float32)
    Wr = np.asarray(Wr, np.float32)
    b = np.asarray(b, np.float32)
    Wd = np.asarray(Wd, np.float32)
    bd = np.asarray(bd, np.float32)
    n_out = int(out_steps)
    n_warm = inputs.shape[1]

    nc = _build(n_warm, n_out)
    in_maps = _prep_core_inputs(inputs, Wk, Wr, b, Wd, bd, n_warm, n_out)
    res = run_bass_kernel_spmd(nc, in_maps, core_ids=list(range(N_CORES)))

    out = np.empty((B_FULL, n_out, F_DIM), np.float32)
    for c in range(N_CORES):
        o = res.results[c]["out"].reshape(F_DIM, n_out, B)       # [F, t, b]
        out[c * B:(c + 1) * B] = o.transpose(2, 1, 0)
    return out
